# revision 21
# baseline (speedup 1.0000x reference)
"""Trainium2 Bass kernel for nn_DiscriminatorImg (dense CNN discriminator).

Strategy: pure data parallel over 8 NeuronCores, 32 samples per core.

Math notes (exact transformations, not approximations):
  - InstanceNorm subtracts the per-(sample,channel) spatial mean, so any
    per-(sample,channel) spatially-constant additive term cancels exactly.
    Hence conv biases b1..b4 AND the broadcast-label concat contribution to
    conv2 are mathematically dead; only the image-channel part of w2 matters.
  - Every conv is 2x2 stride-2 (non-overlapping patches) == a matmul over
    (in_ch x 4) patch vectors.  Patch layouts are produced directly by the
    previous conv's matmul via block-diagonal weight replication, so no
    on-chip im2col shuffles are ever needed.

Per-core pipeline (all shapes per sample unless noted):
  conv1: K=96 (8 spatial parity classes x 12), M=128 (class x 16oc), N=512
  conv2: K=128 (2 col-classes x 4 patches x 16c), M=64 (class x 32oc), N=512
  conv3: 4 accumulating matmuls K=32, M=64, N=256 (strided rhs APs)
  conv4: 4 accumulating matmuls K=64, M=128, N=256 (4 samples batched)
  fc1:   64 accumulating matmuls K=128, M=32(batch), N=2x512, weights
         streamed from HBM through a 56-deep SBUF prefetch ring (critical
         path; ring DMAs are dependency-free so they start at t=0 and the
         8 un-prefetched chunks hide under the FC matmul stream)
  fc2:   single DVE multiply-reduce against a replicated weight row
"""

import sys

sys.path.insert(0, "/opt/trn_rl_repo")

import numpy as np

import concourse.bass as bass
import concourse.bacc as bacc
import concourse.mybir as mybir
import concourse.tile as tile
from concourse.bass_utils import run_bass_kernel_spmd

F32 = mybir.dt.float32
BF16 = mybir.dt.float16  # fp16: same speed as bf16, 3 more mantissa bits
AF = mybir.ActivationFunctionType
ALU = mybir.AluOpType
AX = mybir.AxisListType

NCORES = 8
B = 256
PC = B // NCORES  # 32 samples per core
EPS = 1e-5

_CACHE = {}


def _af():
    # CoreSim has no Lrelu; sim runs substitute Relu (mirrored in sim's
    # numpy model). Hardware always uses Lrelu.
    return AF.Relu if _CACHE.get("sim_relu") else AF.Lrelu


# ---------------------------------------------------------------- host packing
def _pack_x(x):
    """[B,3,128,128] -> [B,96,512] conv1 patch layout.

    partition = cls*12 + c*4 + a*2 + bb   (cls = v*4 + a2*2 + b2)
    free      = i2*16 + u
    value     = x[c, 2*(2*i2+a2)+a, 2*(4*u+2*v+b2)+bb]
    """
    n = x.shape[0]
    xp = x.reshape(n, 3, 32, 2, 2, 16, 2, 2, 2)  # [B,c,i2,a2,a,u,v,b2,bb]
    xp = xp.transpose(0, 6, 3, 7, 1, 4, 8, 2, 5)  # [B,v,a2,b2,c,a,bb,i2,u]
    return np.ascontiguousarray(
        xp.reshape(n, 96, 512).astype(np.float16))


def _pack_weights(w1, w2, w3, w4, fcw1, fcb1, fcw2, fcb2):
    f32 = np.float32
    # conv1 block-diagonal [96,128]
    w1f = w1.reshape(16, 12).astype(f32)  # [o,(c,a,bb)]
    w1blk = np.zeros((96, 128), f32)
    for cls in range(8):
        w1blk[cls * 12:(cls + 1) * 12, cls * 16:(cls + 1) * 16] = w1f.T
    # conv2: image channels only, block-diagonal over v  [128,64]
    w2i = w2[:, :16].astype(f32)  # [32,16,2,2]
    blk = w2i.transpose(2, 3, 1, 0).reshape(64, 32)  # [(a2,b2,c1),o2]
    w2blk = np.zeros((128, 64), f32)
    for v in range(2):
        w2blk[v * 64:(v + 1) * 64, v * 32:(v + 1) * 32] = blk
    # conv3: single-pass K=128 = (a3, v, c2); lhsT [128, 64 oc3]
    # rows a3*64 + v*32 + c2 = w3[oc3, c2, kh=a3, kw=v]
    w3n = np.zeros((128, 64), f32)
    for a3 in range(2):
        for v in range(2):
            w3n[a3 * 64 + v * 32:a3 * 64 + v * 32 + 32, :] = \
                w3[:, :, a3, v].astype(f32).T
    # conv4: two accumulating passes over b4; K=128 = (a4, c3), M=128 oc4
    # w4n[a4*64 + c3, b4, oc4] = w4[oc4, c3, a4, b4]
    w4n = np.zeros((128, 2, 128), f32)
    for a4 in range(2):
        for b4 in range(2):
            w4n[a4 * 64:(a4 + 1) * 64, b4, :] = w4[:, :, a4, b4].astype(f32).T
    # fc1 weights: [1024,8192] -> [oc-half=2, s=64, c4=128, 512], fp16.
    # Half-major so the SBUF ring streams all of half 0 before half 1.
    w1rf = fcw1.reshape(1024, 128, 64).transpose(2, 1, 0).astype(np.float16)
    w1r = np.ascontiguousarray(
        w1rf.reshape(64, 128, 2, 512).transpose(2, 0, 1, 3))
    # broadcast/sum matrices for cross-partition IN stats
    p = np.arange(128)
    b1m = (p[:, None] % 16 == p[None, :] % 16).astype(f32)
    b2m = (p[:, None] % 32 == p[None, :] % 32).astype(f32)
    b3m = (p[:, None] % 64 == p[None, :] % 64).astype(f32)
    fcb1r = np.ascontiguousarray(np.broadcast_to(fcb1.astype(f32), (PC, 1024)))
    fcw2r = np.ascontiguousarray(np.broadcast_to(fcw2.astype(f32)[0], (PC, 1024)))
    fcb2r = np.full((PC, 1), float(np.asarray(fcb2).reshape(-1)[0]), f32)
    bf = np.float16
    return dict(w1blk=w1blk.astype(bf), w2blk=w2blk.astype(bf),
                w3n=w3n.astype(bf), w4n=w4n.astype(bf), w1r=w1r,
                b1m=b1m, b2m=b2m, b3m=b3m,
                fcb1r=fcb1r, fcw2r=fcw2r, fcb2r=fcb2r)


# ------------------------------------------------------------- device program
def _build(level=100, repeat=1):
    # level: 1..4 = conv-stage prefixes (debug), 100 = full kernel.
    ring_n = 64 if level >= 100 else 0
    fc_n = 64 if level >= 100 else 0
    nc = bacc.Bacc("TRN2", target_bir_lowering=False, debug=False,
                   num_devices=NCORES)

    xp_d = nc.dram_tensor("xp", [PC, 96, 512], BF16, kind="ExternalInput")
    w1b_d = nc.dram_tensor("w1blk", [96, 128], BF16, kind="ExternalInput")
    w2b_d = nc.dram_tensor("w2blk", [128, 64], BF16, kind="ExternalInput")
    w3n_d = nc.dram_tensor("w3n", [128, 64], BF16, kind="ExternalInput")
    w4n_d = nc.dram_tensor("w4n", [128, 2, 128], BF16, kind="ExternalInput")
    w1r_d = nc.dram_tensor("w1r", [2, 64, 128, 512], BF16, kind="ExternalInput")
    b1m_d = nc.dram_tensor("b1m", [128, 128], F32, kind="ExternalInput")
    b2m_d = nc.dram_tensor("b2m", [128, 128], F32, kind="ExternalInput")
    b3m_d = nc.dram_tensor("b3m", [128, 128], F32, kind="ExternalInput")
    fcb1_d = nc.dram_tensor("fcb1r", [PC, 1024], F32, kind="ExternalInput")
    fcw2_d = nc.dram_tensor("fcw2r", [PC, 1024], F32, kind="ExternalInput")
    fcb2_d = nc.dram_tensor("fcb2r", [PC, 1], F32, kind="ExternalInput")
    y_d = nc.dram_tensor("y", [PC, 1], F32, kind="ExternalOutput")
    dbg_d = (nc.dram_tensor("dbg", [128, 2048], F32, kind="ExternalOutput")
             if level < 100 else None)

    with tile.TileContext(nc) as tc:
        with (
            tc.tile_pool(name="cw", bufs=1) as cw,
            tc.tile_pool(name="xpp", bufs=8) as xpp,
            tc.tile_pool(name="p2p", bufs=4) as p2p,
            tc.tile_pool(name="h2p", bufs=4) as h2p,
            tc.tile_pool(name="h3p", bufs=2) as h3p,
            tc.tile_pool(name="h4p", bufs=1) as h4p,
            tc.tile_pool(name="sqp", bufs=2) as sqp,
            tc.tile_pool(name="stp", bufs=4) as stp,
            tc.tile_pool(name="wrp", bufs=100) as wrp,
            tc.tile_pool(name="fcp", bufs=1) as fcp,
        ):
            # constants
            w1b = cw.tile([96, 128], BF16)
            nc.sync.dma_start(w1b[:], w1b_d[:])
            w2b = cw.tile([128, 64], BF16)
            nc.sync.dma_start(w2b[:], w2b_d[:])
            w3t = cw.tile([128, 64], BF16)
            nc.sync.dma_start(w3t[:], w3n_d[:])
            w4t = cw.tile([128, 2, 128], BF16)
            nc.sync.dma_start(w4t[:], w4n_d[:])
            b1t = cw.tile([128, 128], F32)
            nc.sync.dma_start(b1t[:], b1m_d[:])
            b2t = cw.tile([128, 128], F32)
            nc.sync.dma_start(b2t[:], b2m_d[:])
            b3t = cw.tile([128, 128], F32)
            nc.sync.dma_start(b3t[:], b3m_d[:])
            fb1 = cw.tile([PC, 1024], F32)
            nc.sync.dma_start(fb1[:], fcb1_d[:])
            fw2 = cw.tile([PC, 1024], F32)
            nc.sync.dma_start(fw2[:], fcw2_d[:])
            fb2 = cw.tile([PC, 1], F32)
            nc.sync.dma_start(fb2[:], fcb2_d[:])

            for _rep in range(repeat):
                h4n = h4p.tile([128, PC, 64], BF16)  # fc1 activations, all samples

                epst = cw.tile([128, 1], F32)
                nc.vector.memset(epst[:], EPS)
                zbt = cw.tile([128, 1], F32)
                nc.vector.memset(zbt[:], 0.0)

                # fc1 weight stream: issue all ring DMAs up front; they are
                # dependency-free so the first `wrp.bufs` start at t=0 and
                # stream continuously under the conv phase.
                wtiles = []
                for h2 in range(2 if ring_n else 0):
                    for s in range(ring_n):
                        wt = wrp.tile([128, 512], BF16, tag="w1ring")
                        nc.gpsimd.dma_start(wt[:], w1r_d[h2, s])
                        wtiles.append(wt)

                with (
                    tc.tile_pool(name="ps1", bufs=2, space="PSUM") as ps1,
                    tc.tile_pool(name="ps2", bufs=2, space="PSUM") as ps2,
                    tc.tile_pool(name="ps3", bufs=2, space="PSUM") as ps3,
                    tc.tile_pool(name="psm", bufs=2, space="PSUM") as psm,
                ):
                    for g in range(8):
                        if level < 4 and g > 0:
                            break
                        # h3g free = (sample-in-group, i4, j3)
                        h3g = h3p.tile([128, 4, 8, 16], BF16)
                        for pr in range(2):
                            if level < 3 and pr > 0:
                                break
                            # ---- conv1 (one matmul per sample) ----
                            p1s = []
                            for h in range(2):
                                b = g * 4 + pr * 2 + h
                                xt = xpp.tile([96, 512], BF16, tag="xt")
                                nc.sync.dma_start(xt[:], xp_d[b])
                                p1 = ps1.tile([128, 512], F32, tag="p1")
                                nc.tensor.matmul(p1[:], w1b[:], xt[:])
                                p1s.append(p1)
                            # ---- IN1 stats: per-partition s1/s2, then sum over the
                            #      8 class-partitions + broadcast via b1t matmul ----
                            s_in = stp.tile([128, 4], F32, tag="s_in")
                            for h in range(2):
                                sqt = sqp.tile([128, 512], F32, tag="sqt")
                                nc.scalar.activation(sqt[:], p1s[h][:], AF.Square,
                                                     accum_out=s_in[:, 2 * h + 1:2 * h + 2])
                                nc.vector.reduce_sum(s_in[:, 2 * h:2 * h + 1], p1s[h][:],
                                                     axis=AX.X)
                            pst = psm.tile([128, 4], F32, tag="misc")
                            nc.tensor.matmul(pst[:], b1t[:], s_in[:])
                            t1 = stp.tile([128, 2], F32, tag="t1")
                            nc.vector.tensor_scalar_mul(t1[:], pst[:, 0::2],
                                                        1.0 / 4096.0)
                            m1 = stp.tile([128, 2], F32, tag="m1")
                            nc.vector.tensor_mul(m1[:], t1[:], t1[:])
                            v1 = stp.tile([128, 2], F32, tag="v1")
                            nc.vector.scalar_tensor_tensor(
                                v1[:], pst[:, 1::2], 1.0 / 4096.0, m1[:],
                                op0=ALU.mult, op1=ALU.subtract)
                            sd1 = stp.tile([128, 2], F32, tag="sd1")
                            nc.scalar.activation(sd1[:], v1[:], AF.Sqrt, bias=epst[:, 0:1])
                            r1 = stp.tile([128, 2], F32, tag="r1")
                            nc.vector.reciprocal(r1[:], sd1[:])
                            nm1 = stp.tile([128, 2], F32, tag="nm1")
                            nc.vector.scalar_tensor_tensor(
                                nm1[:], t1[:], -1.0, r1[:],
                                op0=ALU.mult, op1=ALU.mult)
                            # ---- norm1 + lrelu -> conv2 patch tiles ----
                            # p2tg free = (sample-in-pair, i2', u)
                            p2tg = p2p.tile([128, 2, 32, 16], BF16, tag="p2t")
                            for h in range(2):
                                nc.scalar.activation(p2tg[:, h], p1s[h][:], _af(),
                                                     bias=nm1[:, h:h + 1],
                                                     scale=r1[:, h:h + 1], alpha=0.01)
                            if level == 1:
                                nc.sync.dma_start(
                                    dbg_d[:, 0:1024],
                                    p2tg.rearrange("p a b c -> p (a b c)")[:])
                                continue
                            # ---- conv2: 2 matmuls split by output-row parity a3;
                            #      out partitions = a3*64 + v*32 + oc2 ----
                            p2g = ps2.tile([128, 2, 16, 16], F32, tag="p2g")
                            for a3 in range(2):
                                nc.tensor.matmul(p2g[a3 * 64:(a3 + 1) * 64], w2b[:],
                                                 p2tg[:, :, a3::2, :])
                            # ---- IN2 stats (sum over a3/v partitions via b2t) ----
                            p2f = p2g.rearrange("p a b c -> p a (b c)")
                            s2in = stp.tile([128, 4], F32, tag="s2in")
                            for h in range(2):
                                sq2 = sqp.tile([128, 256], F32, tag="sq2")
                                nc.scalar.activation(sq2[:], p2f[:, h], AF.Square,
                                                     accum_out=s2in[:, 2 * h + 1:2 * h + 2])
                                nc.vector.reduce_sum(s2in[:, 2 * h:2 * h + 1],
                                                     p2f[:, h], axis=AX.X)
                            pst2 = psm.tile([128, 4], F32, tag="misc")
                            nc.tensor.matmul(pst2[:], b2t[:], s2in[:])
                            t2 = stp.tile([128, 2], F32, tag="t2")
                            nc.vector.tensor_scalar_mul(t2[:], pst2[:, 0::2],
                                                        1.0 / 1024.0)
                            m2 = stp.tile([128, 2], F32, tag="m2")
                            nc.vector.tensor_mul(m2[:], t2[:], t2[:])
                            v2 = stp.tile([128, 2], F32, tag="v2")
                            nc.vector.scalar_tensor_tensor(
                                v2[:], pst2[:, 1::2], 1.0 / 1024.0, m2[:],
                                op0=ALU.mult, op1=ALU.subtract)
                            sd2 = stp.tile([128, 2], F32, tag="sd2")
                            nc.scalar.activation(sd2[:], v2[:], AF.Sqrt, bias=epst[:, 0:1])
                            r2 = stp.tile([128, 2], F32, tag="r2")
                            nc.vector.reciprocal(r2[:], sd2[:])
                            nm2 = stp.tile([128, 2], F32, tag="nm2")
                            nc.vector.scalar_tensor_tensor(
                                nm2[:], t2[:], -1.0, r2[:],
                                op0=ALU.mult, op1=ALU.mult)
                            # ---- norm2 + lrelu -> h2g (sample, i3', u) ----
                            h2g = h2p.tile([128, 2, 16, 16], BF16, tag="h2t")
                            for h in range(2):
                                nc.scalar.activation(h2g[:, h], p2g[:, h], _af(),
                                                     bias=nm2[:, h:h + 1],
                                                     scale=r2[:, h:h + 1], alpha=0.01)
                            if level == 2:
                                nc.sync.dma_start(
                                    dbg_d[:, 0:512],
                                    h2g.rearrange("p a b c -> p (a b c)")[:])
                                continue
                            # ---- conv3: single-pass K=128, split by out-row
                            #      parity a4; out partitions = a4*64 + oc3 ----
                            p3g = ps3.tile([128, 2, 8, 16], F32, tag="p3")
                            for a4 in range(2):
                                nc.tensor.matmul(p3g[a4 * 64:(a4 + 1) * 64], w3t[:],
                                                 h2g[:, :, a4::2, :])
                            # ---- IN3 stats (sum over a4 partition pairs via b3t) ----
                            p3f = p3g.rearrange("p a b c -> p a (b c)")
                            s3in = stp.tile([128, 4], F32, tag="s3in")
                            for h in range(2):
                                sq3 = sqp.tile([128, 128], F32, tag="sq3")
                                nc.scalar.activation(sq3[:], p3f[:, h], AF.Square,
                                                     accum_out=s3in[:, 2 * h + 1:2 * h + 2])
                                nc.vector.reduce_sum(s3in[:, 2 * h:2 * h + 1],
                                                     p3f[:, h], axis=AX.X)
                            pst3 = psm.tile([128, 4], F32, tag="misc")
                            nc.tensor.matmul(pst3[:], b3t[:], s3in[:])
                            t3 = stp.tile([128, 2], F32, tag="t3")
                            nc.vector.tensor_scalar_mul(t3[:], pst3[:, 0::2],
                                                        1.0 / 256.0)
                            m3 = stp.tile([128, 2], F32, tag="m3")
                            nc.vector.tensor_mul(m3[:], t3[:], t3[:])
                            v3 = stp.tile([128, 2], F32, tag="v3")
                            nc.vector.scalar_tensor_tensor(
                                v3[:], pst3[:, 1::2], 1.0 / 256.0, m3[:],
                                op0=ALU.mult, op1=ALU.subtract)
                            sd3 = stp.tile([128, 2], F32, tag="sd3")
                            nc.scalar.activation(sd3[:], v3[:], AF.Sqrt, bias=epst[:, 0:1])
                            r3 = stp.tile([128, 2], F32, tag="r3")
                            nc.vector.reciprocal(r3[:], sd3[:])
                            nm3 = stp.tile([128, 2], F32, tag="nm3")
                            nc.vector.scalar_tensor_tensor(
                                nm3[:], t3[:], -1.0, r3[:],
                                op0=ALU.mult, op1=ALU.mult)
                            for h in range(2):
                                nc.scalar.activation(h3g[:, pr * 2 + h], p3g[:, h],
                                                     _af(), bias=nm3[:, h:h + 1],
                                                     scale=r3[:, h:h + 1], alpha=0.01)
                        if level == 3:
                            nc.sync.dma_start(
                                dbg_d[:, 0:512],
                                h3g.rearrange("p a b c -> p (a b c)")[:])
                            continue
                        # ---- conv4: 2 accumulating matmuls over col parity b4;
                        #      K=128 = (a4, c3), M=128 oc4, 4 samples in free ----
                        p4 = psm.tile([128, 4, 8, 8], F32, tag="misc")
                        for b4 in range(2):
                            nc.tensor.matmul(p4[:], w4t[:, b4],
                                             h3g[:, :, :, b4::2],
                                             start=(b4 == 0), stop=(b4 == 1))
                        p4g = p4.rearrange("p a b c -> p a (b c)")
                        s64 = stp.tile([128, 4, 6], F32, tag="s64")
                        mv4 = stp.tile([128, 8], F32, tag="mv4")
                        for k4 in range(4):
                            nc.vector.bn_stats(s64[:, k4], p4g[:, k4])
                            nc.vector.bn_aggr(mv4[:, 2 * k4:2 * k4 + 2], s64[:, k4])
                        sd4 = stp.tile([128, 4], F32, tag="sd4")
                        nc.scalar.activation(sd4[:], mv4[:, 1::2], AF.Sqrt, bias=epst[:, 0:1])
                        r4 = stp.tile([128, 4], F32, tag="r4")
                        nc.vector.reciprocal(r4[:], sd4[:])
                        mr4 = stp.tile([128, 4], F32, tag="mr4")
                        nc.vector.tensor_mul(mr4[:], mv4[:, 0::2], r4[:])
                        nm4 = stp.tile([128, 4], F32, tag="nm4")
                        nc.vector.tensor_scalar_mul(nm4[:], mr4[:], -1.0)
                        for k4 in range(4):
                            b = g * 4 + k4
                            nc.scalar.activation(
                                h4n[:, b],
                                p4[:, k4].rearrange("p a b -> p (a b)"),
                                _af(), bias=nm4[:, k4:k4 + 1],
                                scale=r4[:, k4:k4 + 1], alpha=0.01)
    
                if level == 4:
                    nc.sync.dma_start(
                        dbg_d[:, 0:2048],
                        h4n.rearrange("p a b -> p (a b)")[:, 0:2048])
                # ---- fc1: two oc-halves streamed sequentially so half 0's
                #      epilogue (lrelu + fc2 partial) hides under half 1's
                #      matmul stream.  Bias is seeded into PSUM via a K=1
                #      matmul against a ones-row, so the activation reads
                #      PSUM directly (no DVE bias add on the tail). ----
                if fc_n:
                    psf_cm = tc.tile_pool(name="psf", bufs=1, space="PSUM")
                    psf = psf_cm.__enter__()
                    a5s = []
                    for h2 in range(2):
                        pfc = psf.tile([PC, 512], F32, tag=f"fc{h2}")
                        for s in range(fc_n):
                            nc.tensor.matmul(pfc[:], h4n[:, :, s:s + 1],
                                             wtiles[h2 * fc_n + s][:],
                                             start=(s == 0), stop=(s == fc_n - 1))
                        t5h = fcp.tile([PC, 512], F32, tag=f"t5{h2}")
                        nc.vector.tensor_add(t5h[:], pfc[:],
                                             fb1[:, h2 * 512:(h2 + 1) * 512])
                        h5h = fcp.tile([PC, 512], F32, tag=f"h5{h2}")
                        nc.scalar.activation(h5h[:], t5h[:], _af(),
                                             bias=zbt[0:PC, 0:1], alpha=0.01)
                        sch = fcp.tile([PC, 512], F32, tag=f"sc{h2}")
                        nc.vector.tensor_mul(sch[:], h5h[:],
                                             fw2[:, h2 * 512:(h2 + 1) * 512])
                        a5h = fcp.tile([PC, 1], F32, tag=f"a5{h2}")
                        nc.vector.reduce_sum(a5h[:], sch[:], axis=AX.X)
                        a5s.append(a5h)
                    asum = fcp.tile([PC, 1], F32, tag="asum")
                    nc.vector.tensor_add(asum[:], a5s[0][:], a5s[1][:])
                    ob = fcp.tile([PC, 1], F32)
                    nc.scalar.activation(ob[:], asum[:], AF.Identity,
                                         bias=fb2[:, 0:1])
                    nc.sync.dma_start(y_d[:], ob[:])

                    psf_cm.__exit__(None, None, None)

    nc.compile()
    return nc


def _get_nc():
    if "nc" not in _CACHE:
        _CACHE["nc"] = _build()
    return _CACHE["nc"]


def prepare_in_maps(inputs):
    x = np.asarray(inputs["x"], np.float32)
    w = _pack_weights(
        np.asarray(inputs["w1"], np.float32), np.asarray(inputs["w2"], np.float32),
        np.asarray(inputs["w3"], np.float32), np.asarray(inputs["w4"], np.float32),
        np.asarray(inputs["fcw1"], np.float32), np.asarray(inputs["fcb1"], np.float32),
        np.asarray(inputs["fcw2"], np.float32), np.asarray(inputs["fcb2"], np.float32))
    xp = _pack_x(x)
    in_maps = []
    for c in range(NCORES):
        m = {"xp": xp[c * PC:(c + 1) * PC]}
        m.update({"w1blk": w["w1blk"], "w2blk": w["w2blk"], "w3n": w["w3n"],
                  "w4n": w["w4n"], "w1r": w["w1r"], "b1m": w["b1m"],
                  "b2m": w["b2m"], "b3m": w["b3m"], "fcb1r": w["fcb1r"],
                  "fcw2r": w["fcw2r"], "fcb2r": w["fcb2r"]})
        in_maps.append(m)
    return in_maps


def _run(inputs, trace=False):
    in_maps = prepare_in_maps(inputs)
    nc = _get_nc()
    res = run_bass_kernel_spmd(nc, in_maps, list(range(NCORES)), trace=trace)
    out = np.concatenate([res.results[c]["y"] for c in range(NCORES)], axis=0)
    return out.astype(np.float32), res


def kernel(**inputs):
    out, _ = _run(inputs)
    return out

